# revision 35
# baseline (speedup 1.0000x reference)
"""Trainium2 Bass kernel for nn_Canvas_by_Distance (vq_codebook).

Math: the reference's StraightThroughSoftMax forward is numerically
hard one-hot(argmax of squared distances), so the output is
    out = nearest_upsample_4x( palette[argmax_c ||sigmoid(w) - p_c||^2] )

Key input-adaptive optimization (computed on host, baked at build):
sigmoid(weight) lives in a tight per-channel box [lo, hi].  For two
colors, dist_{c'}(w) - dist_c(w) is LINEAR in w, so "c' always beats c
on the box" is certified by checking the 8 box corners.  Colors that
are dominated can never be the argmax and are pruned; typically only
K ~ 3 of 16 survive, collapsing the per-pixel work.

The output is written as uint8 (round(color*255); the host gather
decodes /255).  The palette colors survive 8-bit quantization at
~7e-3 global relative error, far inside the 2e-2 gate, and the
dominant HBM write traffic drops 4x vs fp32.

Device algorithm per core (canvas rows sharded 8-ways, 128 rows/core),
pipelined over column chunks:
  - one DMA per load-group of chunks (all 3 channels), sigmoid per
    chunk on ACT
  - per surviving color (j ascending), fused custom-DVE ops:
        u   = (w0-p0)^2 + (w1-p1)^2                      (CBD_SQD2)
        s_q = i32(min(((w2-p2)^2 + u) * QSCALE, CLAMP))  (CBD_SQDA)
        pk  = i32(max(s_q*B + (B-1-j), pk_prev))         (CBD_PKMX)
    For K <= 4 the payload is 2 bits (B=4) and QSCALE caps s_q < 2^22,
    so every CBD_PKMX intermediate (< 2^24) is exact in fp32; payload
    B-1-j makes ties pick the smallest index (matches jnp.argmax).
    For K >= 5 the payload is 4 bits and the tournament falls back to
    tensor_scalar pack + f32-bitcast tensor_max (positive IEEE order
    == int32 order), exact to fp32 precision.
  - v = pk & (B-1), cast f32 by the output converter
  - palette map at CANVAS width: chained custom-DVE 2-values-per-op
    selects (CBD_MAP2) write u16 values byte*257 — i.e. two identical
    u8 bytes, which is the first 2x of the 4x column upsample for free
  - ACT pair-copy (step-0 read AP) doubles u16 elements: the second 2x
  - one output DMA per channel per chunk writes the u16 tile through a
    narrowing u8 bitcast, replicating 4 rows via a step-0 read AP

Palette values / pruning are baked into the instruction stream as
immediates (the kernel is rebuilt per call; inputs are runtime data to
the harness but compile-time constants to the NEFF).
"""

import math
import os

import numpy as np

CH, CW = 1024, 1024          # canvas
OH, OW = 4096, 4096          # image
NCOLORS = 16
NCORES = 8
RPC = CH // NCORES           # canvas rows per core = 128
ORPC = RPC * 4               # output rows per core = 512
# K<=4 tournament: s_q < 2^22 keeps pk = s_q*4 + payload < 2^24 exact in
# fp32 through the CBD_PKMX mul/add/max chain.
QCLAMP22 = float((1 << 22) - 1)
# K>=5 fallback: packed = s_q*16 + 15 must stay <= 0x7F7FFFFF for the
# f32-bitcast max trick.
QCLAMP27 = 133693432.0

# column chunking of the 1024 canvas columns (pipeline compute vs DMA-out);
# LGROUPS batches consecutive chunks into one input DMA (SWDGE gen on the
# Pool engine costs ~1.1us per load, so per-chunk loads pace arrivals too
# slowly during warmup)
CHUNKS = tuple(
    int(x) for x in os.environ.get(
        "CBD_CHUNKS", "128,176,176,192,176,176"
    ).split(",")
)
assert sum(CHUNKS) == CW
LGROUPS = tuple(
    int(x) for x in os.environ.get("CBD_LG", "1,1,2,2").split(",")
)
assert sum(LGROUPS) == len(CHUNKS)
# first WIDE0 chunks map directly into the u16 pair layout (no ACT hop)
WIDE0 = int(os.environ.get("CBD_WIDE", "1"))

_OPS_CACHE = {}
_MODULE_CACHE = {}


def _register_ops():
    """Register the custom DVE ops (idempotent)."""
    if _OPS_CACHE:
        return _OPS_CACHE

    import concourse.dve_ops as dve_ops
    from concourse.dve_spec import (
        C0, C1, C2, One, Spec, Src0, Src1, _has_src1, eq, lower, maxx, minn,
        select, sq,
    )
    from concourse.dve_uop import DveOpSpec

    f32 = np.float32

    def register(name, spec, subdim=False):
        if name in dve_ops._SUB_OPCODE_FOR_NAME:
            return next(o for o in dve_ops.OPS if o.name == name)
        row = dve_ops._CUSTOM_DVE_ROW_BASE + len(dve_ops.OPS)
        assert row < 0x20, "custom DVE opcode rows exhausted"
        dve_ops._SUB_OPCODE_FOR_NAME[name] = row
        shas = {}
        for ver in ("v3", "v4"):
            uops = lower(spec, ver=ver)
            shas[ver] = DveOpSpec(
                name=name, opcode=row, uops=uops, rd1_en=_has_src1(spec)
            ).sha(ver)
        op = dve_ops.DveOp(name, spec, subdim=subdim, uops_sha=shas)
        dve_ops.OPS.append(op)
        dve_ops.CUSTOM_DVE_SPECS[name] = spec
        return op

    _OPS_CACHE["SQD2"] = register(
        "CBD_SQD2",
        Spec(
            body=sq(Src0 - C0) + sq(Src1 - C1),
            reference=lambda in0, in1, s0, s1, imm2: np.square(in0 - f32(s0))
            + np.square(in1 - f32(s1)),
        ),
    )
    _OPS_CACHE["SQDA"] = register(
        "CBD_SQDA",
        Spec(
            body=minn((sq(Src0 - C0) + Src1) * C1, C2),
            reference=lambda in0, in1, s0, s1, imm2: np.minimum(
                (np.square(in0 - f32(s0)) + in1) * f32(s1), f32(imm2)
            ),
        ),
    )
    # pk = max(in0*C1 + payload, in1); caller keeps all values < 2^24 so
    # the fp32 mul/add/max chain is exact
    _OPS_CACHE["PKMX"] = register(
        "CBD_PKMX",
        Spec(
            body=maxx(Src0 * C1 + C0, Src1),
            reference=lambda in0, in1, s0, s1, imm2: np.maximum(
                np.asarray(in0, f32) * f32(s1) + f32(s0),
                np.asarray(in1, f32),
            ),
        ),
    )

    def _map2_ref(in0, in1, s0, s1, imm2):
        in0 = np.asarray(in0, np.float32)
        in1 = np.asarray(in1, np.float32)
        if in1.shape != in0.shape:
            if in1.size == in0.size:  # same elements, different AP shape
                in1 = in1.reshape(in0.shape)
            else:  # [P,1] broadcast Src1
                in1 = in1.reshape(in1.shape[0], *([1] * (in0.ndim - 1)))
        return np.where(
            in0 == f32(s1),
            f32(s0),
            np.where(in0 - f32(1.0) == f32(s1), f32(imm2), in1),
        ).astype(np.float32)

    _OPS_CACHE["MAP2"] = register(
        "CBD_MAP2",
        Spec(
            body=select(eq(Src0, C1), C0, select(eq(Src0 - One, C1), C2, Src1)),
            reference=_map2_ref,
        ),
    )
    return _OPS_CACHE


def _prune_palette(weight, pal):
    """Survivor color indices (ascending) + score upper bound over the box.

    A color c is pruned when some c' strictly dominates it on the whole
    sigmoid(weight) box: dist_{c'} - dist_c is linear in w, so checking
    the 8 corners suffices.  Margins cover host-vs-device sigmoid error.
    """
    wmin = weight.min(axis=(1, 2)).astype(np.float64)
    wmax = weight.max(axis=(1, 2)).astype(np.float64)
    lo = np.clip(1.0 / (1.0 + np.exp(-wmin)) - 1e-4, 0.0, 1.0)
    hi = np.clip(1.0 / (1.0 + np.exp(-wmax)) + 1e-4, 0.0, 1.0)
    corners = np.array(
        [[(lo, hi)[(i >> d) & 1][d] for d in range(3)] for i in range(8)]
    )
    p = pal.astype(np.float64)
    pnorm = (p * p).sum(axis=1)
    dominated = np.zeros(NCOLORS, dtype=bool)
    for c in range(NCOLORS):
        for cp in range(NCOLORS):
            if cp == c:
                continue
            g = -2.0 * corners @ (p[cp] - p[c]) + (pnorm[cp] - pnorm[c])
            if g.min() > 1e-3:
                dominated[c] = True
                break
    surv = [c for c in range(NCOLORS) if not dominated[c]]
    # max possible score over the box (extreme at a corner per color)
    s_ub = float(((corners[:, None, :] - p[None, :, :]) ** 2).sum(-1).max()) * 1.05
    return surv, s_ub


def _quant_params(pal, surv):
    """Per-channel affine u8 quantization over the SURVIVOR color values
    (the only values the output can take): byte = round((v-zero)/scale),
    decode = byte*scale + zero.  Much tighter than a fixed [0,1] grid
    when the survivors' channel values cluster."""
    vals = pal[np.asarray(surv, dtype=np.int64), :].astype(np.float64)
    lo = vals.min(axis=0)
    hi = vals.max(axis=0)
    scale = np.empty(3)
    for d in range(3):
        span = hi[d] - lo[d]
        if span < 1e-9:
            scale[d] = 1e-9
            continue
        # pick the grid pitch span/n (n<=255) that minimizes the worst
        # rounding error over this channel's survivor values — with few
        # distinct values some n puts them all almost exactly on-grid
        rel = vals[:, d] - lo[d]
        best = (np.inf, span / 255.0)
        for nlev in range(1, 256):
            s = span / nlev
            err = np.abs(np.round(rel / s) * s - rel).max()
            if err < best[0]:
                best = (err, s)
        scale[d] = best[1]
    return scale, lo


def _b257(pal, c, d, scale, zero):
    """uint16 value of color (c, d): the u8 byte replicated into both
    byte lanes (b*257), so a u16 element IS two upsampled u8 pixels."""
    b = int(round((float(pal[c, d]) - float(zero[d])) / float(scale[d])))
    return float(min(max(b, 0), 255) * 257)


def _body(tc, nc, out_t, w_t, pal, surv, qscale, qsc, qzero, iters=1):
    """Emit the per-core program; palette/pruning baked as immediates."""
    from contextlib import ExitStack

    import concourse.mybir as mybir

    ops = _register_ops()
    SQD2, SQDA, PKMX, MAP2 = ops["SQD2"], ops["SQDA"], ops["PKMX"], ops["MAP2"]

    f32 = mybir.dt.float32
    i32 = mybir.dt.int32
    u16 = mybir.dt.uint16
    u8 = mybir.dt.uint8
    Act = mybir.ActivationFunctionType
    Alu = mybir.AluOpType

    K = len(surv)
    n = len(CHUNKS)
    w_ap = w_t.ap()                                            # (3, 128, 1024)
    out_r = out_t.ap().rearrange("c (p k) w -> c p k w", k=4)  # (3,128,4,4096)

    # payload width: 2 bits for K<=4 (enables the exact-fp32 fused PKMX
    # tournament) — but its qscale cap of 2^22 flips a handful of
    # near-tie pixels vs the reference, and with the dark survivor
    # palette even ~6 flipped pixels cost ~1e-2 relative error.  The
    # default is therefore the f32-bitcast tournament (qscale 2^27,
    # ~1 flipped pixel); CBD_FUSED=1 trades margin for 2 DVE ops/chunk.
    fused = 2 <= K <= 4 and os.environ.get("CBD_FUSED", "0") == "1"
    vmax = 3 if fused else 15
    qclamp = QCLAMP22 if fused else QCLAMP27

    ctx = ExitStack()
    p_w = ctx.enter_context(tc.tile_pool(name="w", bufs=max(2, len(LGROUPS))))
    p_sg = ctx.enter_context(tc.tile_pool(name="sg", bufs=4))
    p_tmp = ctx.enter_context(tc.tile_pool(name="tmp", bufs=4))
    p_map = ctx.enter_context(tc.tile_pool(name="map", bufs=2))
    p_rep = ctx.enter_context(tc.tile_pool(name="rep", bufs=3))
    p_wide = ctx.enter_context(tc.tile_pool(name="wide", bufs=3))
    p_const = ctx.enter_context(tc.tile_pool(name="const", bufs=1))

    def out_dma(d, col0, F, wide):
        rep_b = wide[:].bitcast(u8).unsqueeze(1).broadcast_to([RPC, 4, 4 * F])
        nc.sync.dma_start(out_r[d, :, :, 4 * col0 : 4 * col0 + 4 * F], rep_b)

    if K == 1:
        for _ in range(iters):
            col0 = 0
            for F in CHUNKS:
                for d in range(3):
                    wide = p_wide.tile([RPC, 2 * F], u16, tag=f"wd{d}")
                    nc.vector.memset(wide[:], _b257(pal, surv[0], d, qsc, qzero))
                    out_dma(d, col0, F, wide)
                col0 += F
        ctx.close()
        return

    # persistent fallback tiles seeding the MAP2 select chains (a [P,1]
    # broadcast Src1 fails on HW; a full 2-D tensor works).  Canvas-width
    # u16, so the memsets are cheap enough to run up front; 2x width so
    # chunk 0's pair-layout wide maps can use them too.
    fbw = []
    for d in range(3):
        t = p_const.tile([RPC, 2 * max(CHUNKS)], u16, tag=f"fbw{d}")
        nc.vector.memset(t[:], _b257(pal, surv[-1], d, qsc, qzero))
        fbw.append(t)

    # chunk index -> (load group index, column offset inside the group)
    c2g = []
    goff = []
    gspan = []  # (col0, Fg) per group
    ci = 0
    col0 = 0
    for gi, ng in enumerate(LGROUPS):
        Fg = sum(CHUNKS[ci : ci + ng])
        gspan.append((col0, Fg))
        off = 0
        for F in CHUNKS[ci : ci + ng]:
            c2g.append(gi)
            goff.append(off)
            off += F
            ci += 1
        col0 += Fg

    v2c = {vmax - j: c for j, c in enumerate(surv)}

    for _ in range(iters):
        # all input loads up front: no data deps, the Pool engine paces
        # descriptor generation.  Group 0 goes via the SP HWDGE ring (no
        # Pool startup memsets, faster generation) to cut the critical
        # path to the first output chunk.
        wts = []
        for gi, (gc0, Fg) in enumerate(gspan):
            wt = p_w.tile([RPC, 3 * Fg], f32, tag=f"w{gi}")
            eng = nc.sync if gi == 0 else nc.gpsimd
            eng.dma_start(
                wt[:].rearrange("p (c f) -> p c f", c=3),
                w_ap[:, :, gc0 : gc0 + Fg].rearrange("c p f -> p c f"),
            )
            wts.append(wt)

        def emit_sig(i):
            F = CHUNKS[i]
            wt = wts[c2g[i]]
            Fg = gspan[c2g[i]][1]
            off = goff[i]
            sgt = p_sg.tile([RPC, 3 * F], f32, tag="sg")
            wt_v = wt[:].rearrange("p (c f) -> p c f", c=3)
            nc.scalar.activation(
                sgt[:].rearrange("p (c f) -> p c f", c=3),
                wt_v[:, :, off : off + F], Act.Sigmoid,
            )
            return sgt

        sg_next = emit_sig(0)
        col0 = 0
        for i, F in enumerate(CHUNKS):
            sgt = sg_next
            sg = [sgt[:, d * F : (d + 1) * F] for d in range(3)]

            # --- scores + packed tournament ------------------------------
            pk = None
            for j, c in enumerate(surv):
                u = p_tmp.tile([RPC, F], f32, tag="u")
                nc.vector._custom_dve(
                    SQD2, out=u[:], in0=sg[0], in1=sg[1],
                    s0=float(pal[c, 0]), s1=float(pal[c, 1]),
                )
                sq_ = p_tmp.tile([RPC, F], i32, tag="sq")
                nc.vector._custom_dve(
                    SQDA, out=sq_[:], in0=sg[2], in1=u[:],
                    s0=float(pal[c, 2]), s1=qscale, imm2=qclamp,
                )
                if fused:
                    nk = p_tmp.tile([RPC, F], i32, tag=f"pk{j % 2}")
                    nc.vector._custom_dve(
                        PKMX, out=nk[:], in0=sq_[:],
                        # j == 0: max(s_q*4+3, s_q) == s_q*4+3 seeds it
                        in1=(pk[:] if pk is not None else sq_[:]),
                        s0=float(vmax - j), s1=float(vmax + 1),
                    )
                    pk = nk
                elif j == 0:
                    pk = p_w.tile([RPC, F], i32, tag="packed")
                    nc.vector.tensor_scalar(
                        pk[:], sq_[:], 4, vmax - j,
                        Alu.arith_shift_left, Alu.bitwise_or,
                    )
                else:
                    cand = p_tmp.tile([RPC, F], i32, tag="cand")
                    nc.vector.tensor_scalar(
                        cand[:], sq_[:], 4, vmax - j,
                        Alu.arith_shift_left, Alu.bitwise_or,
                    )
                    # positive IEEE f32 order == int32 order.  (These max
                    # ops must stay on the DVE: the GPSIMD/Pool engine has
                    # no TensorTensor/TensorScalar opcodes in the V3 ISA —
                    # walrus codegen rejects them, even though the cost
                    # model and CoreSim accept them.)
                    nc.vector.tensor_max(
                        pk[:].bitcast(f32), pk[:].bitcast(f32),
                        cand[:].bitcast(f32),
                    )

            # v = pk & vmax (= vmax - j); bitwise ops can't cast, so idx
            # stays i32 and MAP2 reads it via the DVE input converter
            # (values 0..15 convert exactly to f32)
            idx = p_w.tile([RPC, F], i32, tag="idx")
            nc.vector.tensor_scalar(idx[:], pk[:], vmax, None, Alu.bitwise_and)

            # --- palette map, u16 = byte*257 ------------------------------
            # chunk 0 maps straight into the pair layout at width 2F (in0
            # reads idx through a step-0 broadcast AP): the first output
            # DMA then needs no ACT pair-copy, which would otherwise sit
            # on the critical path behind already-ready sigmoids in the
            # ACT queue.  Later chunks map at width F and pair-copy on ACT.
            wide0 = i < WIDE0
            W = 2 * F if wide0 else F
            in0 = (
                idx[:].unsqueeze(2).broadcast_to([RPC, F, 2]) if wide0
                else idx[:]
            )
            rep16 = []
            for d in range(3):
                r16 = (p_wide if wide0 else p_rep).tile(
                    [RPC, W], u16, tag=(f"wd{d}" if wide0 else f"rep{d}")
                )
                if K <= 3:
                    nc.vector._custom_dve(
                        MAP2, out=r16[:], in0=in0, in1=fbw[d][:, :W],
                        s0=_b257(pal, v2c[vmax - 1], d, qsc, qzero)
                        if vmax - 1 in v2c
                        else _b257(pal, surv[0], d, qsc, qzero),
                        s1=float(vmax - 1),
                        imm2=_b257(pal, v2c[vmax], d, qsc, qzero),
                    )
                else:
                    vlo = vmax + 1 - K - (K % 2)
                    cur = fbw[d][:, :W]
                    for v in range(vlo, vmax + 1, 2):
                        last = v + 2 > vmax
                        nxt = r16 if last else p_map.tile(
                            [RPC, W], f32, tag=f"m{d}"
                        )
                        nc.vector._custom_dve(
                            MAP2, out=nxt[:], in0=in0, in1=cur,
                            s0=_b257(pal, v2c.get(v, surv[-1]), d, qsc, qzero),
                            s1=float(v),
                            imm2=_b257(pal, v2c.get(v + 1, surv[-1]), d, qsc, qzero),
                        )
                        cur = nxt[:]
                rep16.append(r16)

            # next chunk's sigmoid goes on the ACT queue BEFORE this
            # chunk's pair-copies so the DVE never waits on it
            if i + 1 < n:
                sg_next = emit_sig(i + 1)

            # --- ACT pair-copy (2nd 2x) + row-replicating store ----------
            for d in range(3):
                if wide0:
                    out_dma(d, col0, F, rep16[d])
                    continue
                wide = p_wide.tile([RPC, 2 * F], u16, tag=f"wd{d}")
                nc.scalar.copy(
                    wide[:],
                    rep16[d][:].unsqueeze(2).broadcast_to([RPC, F, 2]),
                )
                out_dma(d, col0, F, wide)
            col0 += F

    ctx.close()


def build_module(weight, pal):
    """Build + compile the single-core Bass program (palette baked in)."""
    surv, s_ub = _prune_palette(weight, pal)
    K = len(surv)
    if 2 <= K <= 4 and os.environ.get("CBD_FUSED", "0") == "1":
        qscale = float(2.0 ** min(22, int(math.floor(math.log2(QCLAMP22 / s_ub)))))
    else:
        qscale = float(2.0 ** min(30, int(math.floor(math.log2(QCLAMP27 / s_ub)))))
    iters = int(os.environ.get("CBD_ITERS", "1"))
    key = (pal.astype(np.float32).tobytes(), tuple(surv), qscale, iters,
           CHUNKS, LGROUPS, WIDE0)
    if key in _MODULE_CACHE:
        return _MODULE_CACHE[key]

    import concourse.bacc as bacc
    import concourse.mybir as mybir
    import concourse.tile as tile

    nc = bacc.Bacc("TRN2", target_bir_lowering=False, debug=False)
    w_in = nc.dram_tensor("w", [3, RPC, CW], mybir.dt.float32, kind="ExternalInput")
    out = nc.dram_tensor(
        "out", [3, ORPC, OW], mybir.dt.uint8, kind="ExternalOutput"
    )
    qsc, qzero = _quant_params(pal, surv)
    with tile.TileContext(nc) as tc:
        _body(tc, nc, out, w_in, pal, surv, qscale, qsc, qzero, iters=iters)
    nc.compile()
    nc._cbd_qparams = (qsc, qzero)
    _MODULE_CACHE[key] = nc
    return nc


def decode_out(a, qparams):
    """u8 device output -> f32 colors (per-channel affine dequant)."""
    qsc, qzero = qparams
    s = np.asarray(qsc, np.float32).reshape(3, 1, 1)
    z = np.asarray(qzero, np.float32).reshape(3, 1, 1)
    return np.asarray(a).astype(np.float32) * s + z


def kernel(weight, palette):
    """Full inputs in, full output out. Shards rows across 8 NeuronCores."""
    from concourse.bass_utils import run_bass_kernel_spmd

    weight = np.ascontiguousarray(weight, dtype=np.float32)
    pal = np.ascontiguousarray(palette, dtype=np.float32)
    assert weight.shape == (3, CH, CW) and pal.shape == (NCOLORS, 3)

    nc = build_module(weight, pal)

    in_maps = [
        {"w": np.ascontiguousarray(weight[:, m * RPC : (m + 1) * RPC, :])}
        for m in range(NCORES)
    ]
    trace = bool(int(os.environ.get("CBD_TRACE", "0")))
    res = run_bass_kernel_spmd(
        nc, in_maps, core_ids=list(range(NCORES)), trace=trace
    )
    kernel.last_results = res

    full = np.empty((3, OH, OW), dtype=np.float32)
    for m in range(NCORES):
        full[:, m * ORPC : (m + 1) * ORPC, :] = decode_out(
            res.results[m]["out"], nc._cbd_qparams
        )
    return full


# revision 39
# speedup vs baseline: 1.1013x; 1.1013x over previous
"""Trainium2 Bass kernel for nn_Canvas_by_Distance (vq_codebook).

Math: the reference's StraightThroughSoftMax forward is numerically
hard one-hot(argmax of squared distances), so the output is
    out = nearest_upsample_4x( palette[argmax_c ||sigmoid(w) - p_c||^2] )

Key input-adaptive optimization (computed on host, baked at build):
sigmoid(weight) lives in a tight per-channel box [lo, hi].  For two
colors, dist_{c'}(w) - dist_c(w) is LINEAR in w, so "c' always beats c
on the box" is certified by checking the 8 box corners.  Colors that
are dominated can never be the argmax and are pruned; typically only
K ~ 3 of 16 survive, collapsing the per-pixel work.

The output is written as uint8 (round(color*255); the host gather
decodes /255).  The palette colors survive 8-bit quantization at
~7e-3 global relative error, far inside the 2e-2 gate, and the
dominant HBM write traffic drops 4x vs fp32.

Device algorithm per core (canvas rows sharded 8-ways, 128 rows/core),
pipelined over column chunks:
  - one DMA per load-group of chunks (all 3 channels), sigmoid per
    chunk on ACT
  - per surviving color (j ascending), fused custom-DVE ops:
        u   = (w0-p0)^2 + (w1-p1)^2                      (CBD_SQD2)
        s_q = i32(min(((w2-p2)^2 + u) * QSCALE, CLAMP))  (CBD_SQDA)
        pk  = i32(max(s_q*B + (B-1-j), pk_prev))         (CBD_PKMX)
    For K <= 4 the payload is 2 bits (B=4) and QSCALE caps s_q < 2^22,
    so every CBD_PKMX intermediate (< 2^24) is exact in fp32; payload
    B-1-j makes ties pick the smallest index (matches jnp.argmax).
    For K >= 5 the payload is 4 bits and the tournament falls back to
    tensor_scalar pack + f32-bitcast tensor_max (positive IEEE order
    == int32 order), exact to fp32 precision.
  - v = pk & (B-1), cast f32 by the output converter
  - palette map at CANVAS width: chained custom-DVE 2-values-per-op
    selects (CBD_MAP2) write u16 values byte*257 — i.e. two identical
    u8 bytes, which is the first 2x of the 4x column upsample for free
  - ACT pair-copy (step-0 read AP) doubles u16 elements: the second 2x
  - one output DMA per channel per chunk writes the u16 tile through a
    narrowing u8 bitcast, replicating 4 rows via a step-0 read AP

Palette values / pruning are baked into the instruction stream as
immediates (the kernel is rebuilt per call; inputs are runtime data to
the harness but compile-time constants to the NEFF).
"""

import math
import os

import numpy as np

CH, CW = 1024, 1024          # canvas
OH, OW = 4096, 4096          # image
NCOLORS = 16
NCORES = 8
RPC = CH // NCORES           # canvas rows per core = 128
ORPC = RPC * 4               # output rows per core = 512
# K<=4 tournament: s_q < 2^22 keeps pk = s_q*4 + payload < 2^24 exact in
# fp32 through the CBD_PKMX mul/add/max chain.
QCLAMP22 = float((1 << 22) - 1)
# K>=5 fallback: packed = s_q*16 + 15 must stay <= 0x7F7FFFFF for the
# f32-bitcast max trick.
QCLAMP27 = 133693432.0

# column chunking of the 1024 canvas columns (pipeline compute vs DMA-out);
# LGROUPS batches consecutive chunks into one input DMA (SWDGE gen on the
# Pool engine costs ~1.1us per load, so per-chunk loads pace arrivals too
# slowly during warmup)
CHUNKS = tuple(
    int(x) for x in os.environ.get(
        "CBD_CHUNKS", "128,176,176,192,176,176"
    ).split(",")
)
assert sum(CHUNKS) == CW
LGROUPS = tuple(
    int(x) for x in os.environ.get("CBD_LG", "1,1,2,2").split(",")
)
assert sum(LGROUPS) == len(CHUNKS)
# first WIDE0 chunks map directly into the u16 pair layout (no ACT hop)
WIDE0 = int(os.environ.get("CBD_WIDE", "1"))

_OPS_CACHE = {}
_MODULE_CACHE = {}


def _register_ops():
    """Register the custom DVE ops (idempotent)."""
    if _OPS_CACHE:
        return _OPS_CACHE

    import concourse.dve_ops as dve_ops
    from concourse.dve_spec import (
        C0, C1, C2, One, Spec, Src0, Src1, _has_src1, eq, lower, maxx, minn,
        select, sq,
    )
    from concourse.dve_uop import DveOpSpec

    f32 = np.float32

    def register(name, spec, subdim=False):
        if name in dve_ops._SUB_OPCODE_FOR_NAME:
            return next(o for o in dve_ops.OPS if o.name == name)
        row = dve_ops._CUSTOM_DVE_ROW_BASE + len(dve_ops.OPS)
        assert row < 0x20, "custom DVE opcode rows exhausted"
        dve_ops._SUB_OPCODE_FOR_NAME[name] = row
        shas = {}
        for ver in ("v3", "v4"):
            uops = lower(spec, ver=ver)
            shas[ver] = DveOpSpec(
                name=name, opcode=row, uops=uops, rd1_en=_has_src1(spec)
            ).sha(ver)
        op = dve_ops.DveOp(name, spec, subdim=subdim, uops_sha=shas)
        dve_ops.OPS.append(op)
        dve_ops.CUSTOM_DVE_SPECS[name] = spec
        return op

    _OPS_CACHE["SQD2"] = register(
        "CBD_SQD2",
        Spec(
            body=sq(Src0 - C0) + sq(Src1 - C1),
            reference=lambda in0, in1, s0, s1, imm2: np.square(in0 - f32(s0))
            + np.square(in1 - f32(s1)),
        ),
    )
    _OPS_CACHE["SQDA"] = register(
        "CBD_SQDA",
        Spec(
            body=minn((sq(Src0 - C0) + Src1) * C1, C2),
            reference=lambda in0, in1, s0, s1, imm2: np.minimum(
                (np.square(in0 - f32(s0)) + in1) * f32(s1), f32(imm2)
            ),
        ),
    )
    # pk = max(in0*C1 + payload, in1); caller keeps all values < 2^24 so
    # the fp32 mul/add/max chain is exact
    _OPS_CACHE["PKMX"] = register(
        "CBD_PKMX",
        Spec(
            body=maxx(Src0 * C1 + C0, Src1),
            reference=lambda in0, in1, s0, s1, imm2: np.maximum(
                np.asarray(in0, f32) * f32(s1) + f32(s0),
                np.asarray(in1, f32),
            ),
        ),
    )

    import concourse.dve_spec as _ds
    from concourse.dve_spec import MaxNeg, Zero

    def lt(a, b):
        return _ds.Bin(_ds.AluOp.IS_LT, a, b)

    FMIN = float(np.finfo(np.float32).min)

    # g = (in0 + imm2)*s0 + in1*s1 — one leg of a pairwise linear
    # discriminant dist_a - dist_b (linear in sigma); imm2 folds the
    # |p_a|^2-|p_b|^2 constant into whichever leg has the largest coef
    _OPS_CACHE["LINF"] = register(
        "CBD_LINF",
        Spec(
            body=(Src0 + C2) * C0 + Src1 * C1,
            reference=lambda in0, in1, s0, s1, imm2: (
                (np.asarray(in0, f32) + f32(imm2)) * f32(s0)
                + np.asarray(in1, f32) * f32(s1)
            ),
        ),
    )
    # A = (Src0 < 0) ? -inf : Src1 — "color 1 wins iff A >= 0"
    _OPS_CACHE["SELA"] = register(
        "CBD_SELA",
        Spec(
            body=select(lt(Src0, Zero), MaxNeg, Src1),
            reference=lambda in0, in1, s0, s1, imm2: np.where(
                np.asarray(in0, f32) < 0, f32(FMIN), np.asarray(in1, f32)
            ).astype(f32),
        ),
    )
    # 3-way pick: Src0>=0 -> C0; else Src1>=0 -> C1; else C2
    _OPS_CACHE["SEL3"] = register(
        "CBD_SEL3",
        Spec(
            body=select(
                lt(Src0, Zero), select(lt(Src1, Zero), C2, C1), C0
            ),
            reference=lambda in0, in1, s0, s1, imm2: np.where(
                np.asarray(in0, f32) < 0,
                np.where(np.asarray(in1, f32) < 0, f32(imm2), f32(s1)),
                f32(s0),
            ).astype(f32),
        ),
    )
    # 2-way pick: Src0>=0 -> C0 else C1 (no Src1)
    _OPS_CACHE["SEL2"] = register(
        "CBD_SEL2",
        Spec(
            body=select(lt(Src0, Zero), C1, C0),
            reference=lambda in0, in1, s0, s1, imm2: np.where(
                np.asarray(in0, f32) < 0, f32(s1), f32(s0)
            ).astype(f32),
        ),
    )

    def _map2_ref(in0, in1, s0, s1, imm2):
        in0 = np.asarray(in0, np.float32)
        in1 = np.asarray(in1, np.float32)
        if in1.shape != in0.shape:
            if in1.size == in0.size:  # same elements, different AP shape
                in1 = in1.reshape(in0.shape)
            else:  # [P,1] broadcast Src1
                in1 = in1.reshape(in1.shape[0], *([1] * (in0.ndim - 1)))
        return np.where(
            in0 == f32(s1),
            f32(s0),
            np.where(in0 - f32(1.0) == f32(s1), f32(imm2), in1),
        ).astype(np.float32)

    _OPS_CACHE["MAP2"] = register(
        "CBD_MAP2",
        Spec(
            body=select(eq(Src0, C1), C0, select(eq(Src0 - One, C1), C2, Src1)),
            reference=_map2_ref,
        ),
    )
    return _OPS_CACHE


def _prune_palette(weight, pal):
    """Survivor color indices (ascending) + score upper bound over the box.

    A color c is pruned when some c' strictly dominates it on the whole
    sigmoid(weight) box: dist_{c'} - dist_c is linear in w, so checking
    the 8 corners suffices.  Margins cover host-vs-device sigmoid error.
    """
    wmin = weight.min(axis=(1, 2)).astype(np.float64)
    wmax = weight.max(axis=(1, 2)).astype(np.float64)
    lo = np.clip(1.0 / (1.0 + np.exp(-wmin)) - 1e-4, 0.0, 1.0)
    hi = np.clip(1.0 / (1.0 + np.exp(-wmax)) + 1e-4, 0.0, 1.0)
    corners = np.array(
        [[(lo, hi)[(i >> d) & 1][d] for d in range(3)] for i in range(8)]
    )
    p = pal.astype(np.float64)
    pnorm = (p * p).sum(axis=1)
    dominated = np.zeros(NCOLORS, dtype=bool)
    for c in range(NCOLORS):
        for cp in range(NCOLORS):
            if cp == c:
                continue
            g = -2.0 * corners @ (p[cp] - p[c]) + (pnorm[cp] - pnorm[c])
            if g.min() > 1e-3:
                dominated[c] = True
                break
    surv = [c for c in range(NCOLORS) if not dominated[c]]
    # max possible score over the box (extreme at a corner per color)
    s_ub = float(((corners[:, None, :] - p[None, :, :]) ** 2).sum(-1).max()) * 1.05
    return surv, s_ub


def _quant_params(pal, surv):
    """Per-channel affine u8 quantization over the SURVIVOR color values
    (the only values the output can take): byte = round((v-zero)/scale),
    decode = byte*scale + zero.  Much tighter than a fixed [0,1] grid
    when the survivors' channel values cluster."""
    vals = pal[np.asarray(surv, dtype=np.int64), :].astype(np.float64)
    lo = vals.min(axis=0)
    hi = vals.max(axis=0)
    scale = np.empty(3)
    for d in range(3):
        span = hi[d] - lo[d]
        if span < 1e-9:
            scale[d] = 1e-9
            continue
        # pick the grid pitch span/n (n<=255) that minimizes the worst
        # rounding error over this channel's survivor values — with few
        # distinct values some n puts them all almost exactly on-grid
        rel = vals[:, d] - lo[d]
        best = (np.inf, span / 255.0)
        for nlev in range(1, 256):
            s = span / nlev
            err = np.abs(np.round(rel / s) * s - rel).max()
            if err < best[0]:
                best = (err, s)
        scale[d] = best[1]
    return scale, lo


def _b257(pal, c, d, scale, zero):
    """uint16 value of color (c, d): the u8 byte replicated into both
    byte lanes (b*257), so a u16 element IS two upsampled u8 pixels."""
    b = int(round((float(pal[c, d]) - float(zero[d])) / float(scale[d])))
    return float(min(max(b, 0), 255) * 257)


def _body(tc, nc, out_t, w_t, pal, surv, qscale, qsc, qzero, iters=1):
    """Emit the per-core program; palette/pruning baked as immediates."""
    from contextlib import ExitStack

    import concourse.mybir as mybir

    ops = _register_ops()
    SQD2, SQDA, PKMX, MAP2 = ops["SQD2"], ops["SQDA"], ops["PKMX"], ops["MAP2"]
    LINF, SELA, SEL3, SEL2 = ops["LINF"], ops["SELA"], ops["SEL3"], ops["SEL2"]

    f32 = mybir.dt.float32
    i32 = mybir.dt.int32
    u16 = mybir.dt.uint16
    u8 = mybir.dt.uint8
    Act = mybir.ActivationFunctionType
    Alu = mybir.AluOpType

    K = len(surv)
    n = len(CHUNKS)
    w_ap = w_t.ap()                                            # (3, 128, 1024)
    out_r = out_t.ap().rearrange("c (p k) w -> c p k w", k=4)  # (3,128,4,4096)

    # payload width: 2 bits for K<=4 (enables the exact-fp32 fused PKMX
    # tournament) — but its qscale cap of 2^22 flips a handful of
    # near-tie pixels vs the reference, and with the dark survivor
    # palette even ~6 flipped pixels cost ~1e-2 relative error.  The
    # default is therefore the f32-bitcast tournament (qscale 2^27,
    # ~1 flipped pixel); CBD_FUSED=1 trades margin for 2 DVE ops/chunk.
    fused = 2 <= K <= 4 and os.environ.get("CBD_FUSED", "0") == "1"
    vmax = 3 if fused else 15
    qclamp = QCLAMP22 if fused else QCLAMP27

    ctx = ExitStack()
    p_w = ctx.enter_context(tc.tile_pool(name="w", bufs=max(2, len(LGROUPS))))
    p_sg = ctx.enter_context(tc.tile_pool(name="sg", bufs=4))
    p_tmp = ctx.enter_context(tc.tile_pool(name="tmp", bufs=4))
    p_map = ctx.enter_context(tc.tile_pool(name="map", bufs=2))
    p_rep = ctx.enter_context(tc.tile_pool(name="rep", bufs=3))
    p_wide = ctx.enter_context(tc.tile_pool(name="wide", bufs=3))
    p_const = ctx.enter_context(tc.tile_pool(name="const", bufs=1))

    def out_dma(d, col0, F, wide):
        rep_b = wide[:].bitcast(u8).unsqueeze(1).broadcast_to([RPC, 4, 4 * F])
        nc.sync.dma_start(out_r[d, :, :, 4 * col0 : 4 * col0 + 4 * F], rep_b)

    if K == 1:
        for _ in range(iters):
            col0 = 0
            for F in CHUNKS:
                for d in range(3):
                    wide = p_wide.tile([RPC, 2 * F], u16, tag=f"wd{d}")
                    nc.vector.memset(wide[:], _b257(pal, surv[0], d, qsc, qzero))
                    out_dma(d, col0, F, wide)
                col0 += F
        ctx.close()
        return

    # persistent fallback tiles seeding the MAP2 select chains (a [P,1]
    # broadcast Src1 fails on HW; a full 2-D tensor works).  Canvas-width
    # u16, so the memsets are cheap enough to run up front; 2x width so
    # chunk 0's pair-layout wide maps can use them too.
    fbw = []
    for d in range(3):
        t = p_const.tile([RPC, 2 * max(CHUNKS)], u16, tag=f"fbw{d}")
        nc.vector.memset(t[:], _b257(pal, surv[-1], d, qsc, qzero))
        fbw.append(t)

    # chunk index -> (load group index, column offset inside the group)
    c2g = []
    goff = []
    gspan = []  # (col0, Fg) per group
    ci = 0
    col0 = 0
    for gi, ng in enumerate(LGROUPS):
        Fg = sum(CHUNKS[ci : ci + ng])
        gspan.append((col0, Fg))
        off = 0
        for F in CHUNKS[ci : ci + ng]:
            c2g.append(gi)
            goff.append(off)
            off += F
            ci += 1
        col0 += Fg

    v2c = {vmax - j: c for j, c in enumerate(surv)}

    # pairwise linear discriminants are the cheapest exact argmax for
    # K in {2, 3}: 2 fused ops per pair + 1 sub + 1 guard + direct
    # 3-way color selects — no squared distances, no packing, no maps
    # (except the pair-width chunk-0 path).  Degenerate near-identical
    # color pairs (all |coef| tiny) fall back to the tournament.
    linear = K in (2, 3) and os.environ.get("CBD_LINEAR", "1") == "1"
    pairs = []
    if linear:
        p64 = pal.astype(np.float64)
        for cb in surv[1:]:
            a = -2.0 * (p64[surv[0]] - p64[cb])
            dconst = float((p64[surv[0]] ** 2).sum() - (p64[cb] ** 2).sum())
            if np.abs(a).max() < 1e-6:
                linear = False
            pairs.append((a, dconst))
    vmaxl = K - 1
    v2cl = {vmaxl - j: c for j, c in enumerate(surv)}

    for _ in range(iters):
        # all input loads up front: no data deps, the Pool engine paces
        # descriptor generation.  Group 0 goes via the SP HWDGE ring (no
        # Pool startup memsets, faster generation) to cut the critical
        # path to the first output chunk.
        wts = []
        for gi, (gc0, Fg) in enumerate(gspan):
            wt = p_w.tile([RPC, 3 * Fg], f32, tag=f"w{gi}")
            eng = nc.sync if gi == 0 else nc.gpsimd
            eng.dma_start(
                wt[:].rearrange("p (c f) -> p c f", c=3),
                w_ap[:, :, gc0 : gc0 + Fg].rearrange("c p f -> p c f"),
            )
            wts.append(wt)

        def emit_sig(i):
            F = CHUNKS[i]
            wt = wts[c2g[i]]
            Fg = gspan[c2g[i]][1]
            off = goff[i]
            sgt = p_sg.tile([RPC, 3 * F], f32, tag="sg")
            wt_v = wt[:].rearrange("p (c f) -> p c f", c=3)
            nc.scalar.activation(
                sgt[:].rearrange("p (c f) -> p c f", c=3),
                wt_v[:, :, off : off + F], Act.Sigmoid,
            )
            return sgt

        sg_next = emit_sig(0)
        col0 = 0
        for i, F in enumerate(CHUNKS):
            sgt = sg_next
            sg = [sgt[:, d * F : (d + 1) * F] for d in range(3)]

            wide0 = i < WIDE0
            if linear:
                # --- pairwise linear discriminants ------------------------
                # dist_{c1} - dist_{cb} = sum_k a_k*sigma_k + dconst: two
                # fused ops per pair; the constant folds into the leg with
                # the largest coefficient.  g23 = g13 - g12.  Winner:
                # c1 iff g12>=0 ^ g13>=0 (A = g12<0 ? -inf : g13), else
                # c2 iff g23>=0, else c3 — ties pick the smaller index,
                # matching jnp.argmax.
                gs = []
                for pi, (a, dconst) in enumerate(pairs):
                    h = int(np.argmax(np.abs(a)))
                    t = p_tmp.tile([RPC, F], f32, tag=f"t{pi}")
                    g = p_tmp.tile([RPC, F], f32, tag=f"g{pi}")
                    if h == 2:
                        nc.vector._custom_dve(
                            LINF, out=t[:], in0=sg[0], in1=sg[1],
                            s0=float(a[0]), s1=float(a[1]), imm2=0.0,
                        )
                        nc.vector._custom_dve(
                            LINF, out=g[:], in0=sg[2], in1=t[:],
                            s0=float(a[2]), s1=1.0,
                            imm2=float(dconst / a[2]),
                        )
                    else:
                        o = 1 - h
                        nc.vector._custom_dve(
                            LINF, out=t[:], in0=sg[h], in1=sg[o],
                            s0=float(a[h]), s1=float(a[o]),
                            imm2=float(dconst / a[h]),
                        )
                        nc.vector._custom_dve(
                            LINF, out=g[:], in0=sg[2], in1=t[:],
                            s0=float(a[2]), s1=1.0, imm2=0.0,
                        )
                    gs.append(g)
                if K == 3:
                    g23 = p_tmp.tile([RPC, F], f32, tag="g23")
                    nc.vector.tensor_sub(g23[:], gs[1][:], gs[0][:])
                    ga = p_tmp.tile([RPC, F], f32, tag="ga")
                    nc.vector._custom_dve(
                        SELA, out=ga[:], in0=gs[0][:], in1=gs[1][:]
                    )

                def lin_sel(out_ap, c0, c1v, c2v):
                    if K == 3:
                        nc.vector._custom_dve(
                            SEL3, out=out_ap, in0=ga[:], in1=g23[:],
                            s0=c0, s1=c1v, imm2=c2v,
                        )
                    else:
                        nc.vector._custom_dve(
                            SEL2, out=out_ap, in0=gs[0][:], s0=c0, s1=c1v,
                        )

                rep16 = []
                if wide0:
                    # materialize v (K-1-j codes), then map at pair width
                    v = p_w.tile([RPC, F], f32, tag="idx")
                    lin_sel(v[:], float(vmaxl), float(vmaxl - 1),
                            float(max(vmaxl - 2, 0)))
                    in0v = v[:].unsqueeze(2).broadcast_to([RPC, F, 2])
                    for d in range(3):
                        r16 = p_wide.tile([RPC, 2 * F], u16, tag=f"wd{d}")
                        nc.vector._custom_dve(
                            MAP2, out=r16[:], in0=in0v,
                            in1=fbw[d][:, : 2 * F],
                            s0=_b257(pal, v2cl[vmaxl - 1], d, qsc, qzero),
                            s1=float(vmaxl - 1),
                            imm2=_b257(pal, v2cl[vmaxl], d, qsc, qzero),
                        )
                        rep16.append(r16)
                else:
                    # select the channel color directly at canvas width
                    for d in range(3):
                        r16 = p_rep.tile([RPC, F], u16, tag=f"rep{d}")
                        lin_sel(
                            r16[:],
                            _b257(pal, surv[0], d, qsc, qzero),
                            _b257(pal, surv[1], d, qsc, qzero),
                            _b257(pal, surv[-1], d, qsc, qzero),
                        )
                        rep16.append(r16)
                # next chunk's sigmoid before this chunk's pair-copies
                if i + 1 < n:
                    sg_next = emit_sig(i + 1)
                for d in range(3):
                    if wide0:
                        out_dma(d, col0, F, rep16[d])
                        continue
                    wide = p_wide.tile([RPC, 2 * F], u16, tag=f"wd{d}")
                    nc.scalar.copy(
                        wide[:],
                        rep16[d][:].unsqueeze(2).broadcast_to([RPC, F, 2]),
                    )
                    out_dma(d, col0, F, wide)
                col0 += F
                continue

            # --- scores + packed tournament ------------------------------
            pk = None
            for j, c in enumerate(surv):
                u = p_tmp.tile([RPC, F], f32, tag="u")
                nc.vector._custom_dve(
                    SQD2, out=u[:], in0=sg[0], in1=sg[1],
                    s0=float(pal[c, 0]), s1=float(pal[c, 1]),
                )
                sq_ = p_tmp.tile([RPC, F], i32, tag="sq")
                nc.vector._custom_dve(
                    SQDA, out=sq_[:], in0=sg[2], in1=u[:],
                    s0=float(pal[c, 2]), s1=qscale, imm2=qclamp,
                )
                if fused:
                    nk = p_tmp.tile([RPC, F], i32, tag=f"pk{j % 2}")
                    nc.vector._custom_dve(
                        PKMX, out=nk[:], in0=sq_[:],
                        # j == 0: max(s_q*4+3, s_q) == s_q*4+3 seeds it
                        in1=(pk[:] if pk is not None else sq_[:]),
                        s0=float(vmax - j), s1=float(vmax + 1),
                    )
                    pk = nk
                elif j == 0:
                    pk = p_w.tile([RPC, F], i32, tag="packed")
                    nc.vector.tensor_scalar(
                        pk[:], sq_[:], 4, vmax - j,
                        Alu.arith_shift_left, Alu.bitwise_or,
                    )
                else:
                    cand = p_tmp.tile([RPC, F], i32, tag="cand")
                    nc.vector.tensor_scalar(
                        cand[:], sq_[:], 4, vmax - j,
                        Alu.arith_shift_left, Alu.bitwise_or,
                    )
                    # positive IEEE f32 order == int32 order.  (These max
                    # ops must stay on the DVE: the GPSIMD/Pool engine has
                    # no TensorTensor/TensorScalar opcodes in the V3 ISA —
                    # walrus codegen rejects them, even though the cost
                    # model and CoreSim accept them.)
                    nc.vector.tensor_max(
                        pk[:].bitcast(f32), pk[:].bitcast(f32),
                        cand[:].bitcast(f32),
                    )

            # v = pk & vmax (= vmax - j); bitwise ops can't cast, so idx
            # stays i32 and MAP2 reads it via the DVE input converter
            # (values 0..15 convert exactly to f32)
            idx = p_w.tile([RPC, F], i32, tag="idx")
            nc.vector.tensor_scalar(idx[:], pk[:], vmax, None, Alu.bitwise_and)

            # --- palette map, u16 = byte*257 ------------------------------
            # chunk 0 maps straight into the pair layout at width 2F (in0
            # reads idx through a step-0 broadcast AP): the first output
            # DMA then needs no ACT pair-copy, which would otherwise sit
            # on the critical path behind already-ready sigmoids in the
            # ACT queue.  Later chunks map at width F and pair-copy on ACT.
            wide0 = i < WIDE0
            W = 2 * F if wide0 else F
            in0 = (
                idx[:].unsqueeze(2).broadcast_to([RPC, F, 2]) if wide0
                else idx[:]
            )
            rep16 = []
            for d in range(3):
                r16 = (p_wide if wide0 else p_rep).tile(
                    [RPC, W], u16, tag=(f"wd{d}" if wide0 else f"rep{d}")
                )
                if K <= 3:
                    nc.vector._custom_dve(
                        MAP2, out=r16[:], in0=in0, in1=fbw[d][:, :W],
                        s0=_b257(pal, v2c[vmax - 1], d, qsc, qzero)
                        if vmax - 1 in v2c
                        else _b257(pal, surv[0], d, qsc, qzero),
                        s1=float(vmax - 1),
                        imm2=_b257(pal, v2c[vmax], d, qsc, qzero),
                    )
                else:
                    vlo = vmax + 1 - K - (K % 2)
                    cur = fbw[d][:, :W]
                    for v in range(vlo, vmax + 1, 2):
                        last = v + 2 > vmax
                        nxt = r16 if last else p_map.tile(
                            [RPC, W], f32, tag=f"m{d}"
                        )
                        nc.vector._custom_dve(
                            MAP2, out=nxt[:], in0=in0, in1=cur,
                            s0=_b257(pal, v2c.get(v, surv[-1]), d, qsc, qzero),
                            s1=float(v),
                            imm2=_b257(pal, v2c.get(v + 1, surv[-1]), d, qsc, qzero),
                        )
                        cur = nxt[:]
                rep16.append(r16)

            # next chunk's sigmoid goes on the ACT queue BEFORE this
            # chunk's pair-copies so the DVE never waits on it
            if i + 1 < n:
                sg_next = emit_sig(i + 1)

            # --- ACT pair-copy (2nd 2x) + row-replicating store ----------
            for d in range(3):
                if wide0:
                    out_dma(d, col0, F, rep16[d])
                    continue
                wide = p_wide.tile([RPC, 2 * F], u16, tag=f"wd{d}")
                nc.scalar.copy(
                    wide[:],
                    rep16[d][:].unsqueeze(2).broadcast_to([RPC, F, 2]),
                )
                out_dma(d, col0, F, wide)
            col0 += F

    ctx.close()


def build_module(weight, pal):
    """Build + compile the single-core Bass program (palette baked in)."""
    surv, s_ub = _prune_palette(weight, pal)
    K = len(surv)
    if 2 <= K <= 4 and os.environ.get("CBD_FUSED", "0") == "1":
        qscale = float(2.0 ** min(22, int(math.floor(math.log2(QCLAMP22 / s_ub)))))
    else:
        qscale = float(2.0 ** min(30, int(math.floor(math.log2(QCLAMP27 / s_ub)))))
    iters = int(os.environ.get("CBD_ITERS", "1"))
    key = (pal.astype(np.float32).tobytes(), tuple(surv), qscale, iters,
           CHUNKS, LGROUPS, WIDE0)
    if key in _MODULE_CACHE:
        return _MODULE_CACHE[key]

    import concourse.bacc as bacc
    import concourse.mybir as mybir
    import concourse.tile as tile

    nc = bacc.Bacc("TRN2", target_bir_lowering=False, debug=False)
    w_in = nc.dram_tensor("w", [3, RPC, CW], mybir.dt.float32, kind="ExternalInput")
    out = nc.dram_tensor(
        "out", [3, ORPC, OW], mybir.dt.uint8, kind="ExternalOutput"
    )
    qsc, qzero = _quant_params(pal, surv)
    with tile.TileContext(nc) as tc:
        _body(tc, nc, out, w_in, pal, surv, qscale, qsc, qzero, iters=iters)
    nc.compile()
    nc._cbd_qparams = (qsc, qzero)
    _MODULE_CACHE[key] = nc
    return nc


def decode_out(a, qparams):
    """u8 device output -> f32 colors (per-channel affine dequant)."""
    qsc, qzero = qparams
    s = np.asarray(qsc, np.float32).reshape(3, 1, 1)
    z = np.asarray(qzero, np.float32).reshape(3, 1, 1)
    return np.asarray(a).astype(np.float32) * s + z


def kernel(weight, palette):
    """Full inputs in, full output out. Shards rows across 8 NeuronCores."""
    from concourse.bass_utils import run_bass_kernel_spmd

    weight = np.ascontiguousarray(weight, dtype=np.float32)
    pal = np.ascontiguousarray(palette, dtype=np.float32)
    assert weight.shape == (3, CH, CW) and pal.shape == (NCOLORS, 3)

    nc = build_module(weight, pal)

    in_maps = [
        {"w": np.ascontiguousarray(weight[:, m * RPC : (m + 1) * RPC, :])}
        for m in range(NCORES)
    ]
    trace = bool(int(os.environ.get("CBD_TRACE", "0")))
    res = run_bass_kernel_spmd(
        nc, in_maps, core_ids=list(range(NCORES)), trace=trace
    )
    kernel.last_results = res

    full = np.empty((3, OH, OW), dtype=np.float32)
    for m in range(NCORES):
        full[:, m * ORPC : (m + 1) * ORPC, :] = decode_out(
            res.results[m]["out"], nc._cbd_qparams
        )
    return full
